# revision 151
# baseline (speedup 1.0000x reference)
"""Multi-head causal attention (RoPE + QK-RMSNorm) on 8 TRN2 NeuronCores.

Sharding: data parallel on batch (2) x tensor parallel on heads (4 groups of
2 heads).  core = 4*b + g computes, for batch b, heads [2g, 2g+1]:
  q/k/v projections (E-sliced), qk-rmsnorm + rope, causal attention, and the
  Wo partial product over its E slice.  Host sums the 4 partials per batch.

Everything on device runs in the "transposed" orientation:
  qT/kT [e, s], v [s, e], scoresT [sk, sq], out_T [d_out, s]
so no on-chip data transposes are needed; softmax denominators and rms sums
are computed with ones-matmuls on the TensorEngine in *column* layout
[128 positions, few cols].  1/x uses the DVE divide unit; 1/sqrt(x) uses a
sqrt bit-hack seed + 2 Babylonian iterations + reciprocal, all on DVE, so
the ScalarEngine only ever needs the exp table set (no table thrashing).

fp8 DoubleRow: the projection (x @ Wq/Wk/Wv) and output (aT @ Wo) matmuls
run as fp8e4 DoubleRow pairs over 256-deep contractions.  Each operand is
represented as hi+lo fp8 planes (lo = residual), and the product keeps the
3 significant cross terms (hh, lh, hl), dropping lo*lo: 3 DR matmuls per
256-contraction = 1.5 cycles/col vs 2.0 for bf16 (25% PE savings) at
slightly *better* than bf16 accuracy.  Weights are host-prescaled by 64 to
dodge fp8 subnormals; the scale is undone via RMSNorm for q/k, via the
PSUM->SBUF copy scale for v, and on the host for Wo.  The attention matmuls
(scores, av) stay bf16: 2-terming q/k/e on-chip costs more DVE than it
saves on PE.

Latency lessons baked in (cost model: all DMA transfers serialize through
one DMA_ENGINES device at ~360 GB/s, each DMA also holds the shared HWDGE
~625ns; per-column gpsimd DMAs cost ~1us each on the SWDGE path):
 - col->row transposes go through the PE (permutation matmul vs a host
   identity) + one copy + one SWDGE DMA, not n per-column DMAs.
 - the rmsnorm weights fold into the q/k PSUM-copy scale (per-partition
   scale AP), so the rope tables are pure cos/sin (1MB not 4MB of DMA).
 - q-rope is factored: rotate first (needs only qr), multiply by the
   broadcast rsqrt last, so rope math never waits on the NR chain; the
   rsqrt/denominator broadcast paths run in bf16 (2x DVE).
 - q/k tails are emitted inline (overlap the k/v-section matmuls) at high
   scheduler priority; rsqrt uses 1 Babylonian iteration (0.06% scale err).
 - bulk loads are staggered (wq split, wo deferred past phase_a(1)) so
   latency-critical small DMAs do not queue behind megabytes of weights.
 - av is copied out of PSUM immediately (psb ring slots never wait on the
   denominator chain); x is packed [T,p,c,tt,i,s'] so every load line is
   2KB contiguous; outputs store as bf16, 2 douts per DMA, and the last
   chunk splits stores across HWDGE/SWDGE dispatch paths.
 - a few warm-up matmuls cover the initial DMA wait so the PE p-state ramp
   is done before real work.

Scheduling: the attention inner loop is software-pipelined depth-3 (scores
k+3 issue before av/den k), diagonal tiles compute only their live columns,
rms col-matmuls are hoisted behind each projection section, Wo output
bursts are injected as PE filler into the later attention groups (delayed
3 slots so they never stall on the previous chunk's aT hi/lo chain).

TimelineSim: 242005 ns/core (baseline 308002, -21.4%), rms rel err 8.7e-3.
"""

import math

import numpy as np
import ml_dtypes

import concourse.bass as bass
import concourse.tile as tile
from concourse import bacc, mybir
from concourse.bass_utils import run_bass_kernel_spmd

# Problem shapes (hardcoded per instructions).
B = 2
S = 2048
D = 2048
H = 8
HD = 256
HALF = 128
EL = 512          # E columns per core (2 heads)
CH = 512          # sq chunk size
NCH = S // CH     # 4
DT = D // 128     # 16 k-tiles over D
DP = DT // 2      # 8 k-tile *pairs* over D (DoubleRow)
ET = EL // 128    # 4 e-tiles
EP = ET // 2      # 2 e-tile pairs (DoubleRow Wo contraction)
ST = S // 128     # 16 s-tiles
EPS = 1e-6
N_CORES = 8
WSCALE = 64.0     # host pre-scale on Wq/Wk/Wv/Wo to avoid fp8 subnormals

BF16 = mybir.dt.bfloat16
F32 = mybir.dt.float32
FP8 = mybir.dt.float8e4
I32 = mybir.dt.int32
NBF = ml_dtypes.bfloat16
NF8 = ml_dtypes.float8_e4m3
DR = mybir.MatmulPerfMode.DoubleRow

SQRT_MAGIC = 0x1FBD1DF5     # sqrt(x) seed: (bits(x) >> 1) + MAGIC

_CACHE: dict = {}


def _build(reps: int = 1):
    nc = bacc.Bacc("TRN2", target_bir_lowering=False, debug=False,
                   num_devices=N_CORES)

    # x pairs-of-pairs: [T, p, c, tt, i, s'] = xT[512T+256tt+128i+p, c*CH+s']
    # so each (T, p, c) line is 2KB contiguous in DRAM.
    xh_d = nc.dram_tensor("xh", [DP // 2, 128, NCH, 2, 2, CH], FP8,
                          kind="ExternalInput").ap()
    xl_d = nc.dram_tensor("xl", [DP // 2, 128, NCH, 2, 2, CH], FP8,
                          kind="ExternalInput").ap()
    wq_d = {p: nc.dram_tensor(f"wq{p}", [DP, 128, 2, EL], FP8,
                              kind="ExternalInput").ap() for p in "hl"}
    wk_d = {p: nc.dram_tensor(f"wk{p}", [DP, 128, 2, EL], FP8,
                              kind="ExternalInput").ap() for p in "hl"}
    wv_d = {p: nc.dram_tensor(f"wv{p}", [DP, 128, 2, EL], FP8,
                              kind="ExternalInput").ap() for p in "hl"}
    wo_d = {p: nc.dram_tensor(f"wo{p}", [EP, 128, 2, D], FP8,
                              kind="ExternalInput").ap() for p in "hl"}
    id_d = nc.dram_tensor("ident", [128, 128], F32, kind="ExternalInput").ap()
    idb_d = nc.dram_tensor("identb", [128, 128], BF16, kind="ExternalInput").ap()
    rt_d = nc.dram_tensor("rt", [2, HALF, S], BF16, kind="ExternalInput").ap()
    wn_d = nc.dram_tensor("wn", [128, 4], F32, kind="ExternalInput").ap()
    msk_d = nc.dram_tensor("masks", [4, HALF, CH], BF16, kind="ExternalInput").ap()
    out_d = nc.dram_tensor("outT", [D, S], BF16, kind="ExternalOutput").ap()

    with tile.TileContext(nc) as tc:
        for _ in range(reps):
            _emit(tc, nc, xh_d, xl_d, wq_d, wk_d, wv_d, wo_d, id_d, idb_d,
                  rt_d, wn_d, msk_d, out_d)
    nc.compile()
    return nc


def _emit(tc, nc, xh_d, xl_d, wq_d, wk_d, wv_d, wo_d, id_d, idb_d, rt_d,
          wn_d, msk_d, out_d):
    from contextlib import ExitStack
    ctx = ExitStack()
    with ctx:
        persist = ctx.enter_context(tc.tile_pool(name="persist", bufs=1))
        xs_p = ctx.enter_context(tc.tile_pool(name="xs", bufs=16))
        rt_p = ctx.enter_context(tc.tile_pool(name="rt", bufs=2))
        sq_p = ctx.enter_context(tc.tile_pool(name="sq", bufs=4))
        qr_p = ctx.enter_context(tc.tile_pool(name="qr", bufs=9))
        qs_p = ctx.enter_context(tc.tile_pool(name="qs", bufs=6))
        rtmp_p = ctx.enter_context(tc.tile_pool(name="rtmp", bufs=4))
        rb_p = ctx.enter_context(tc.tile_pool(name="rb", bufs=2))
        rd_p = ctx.enter_context(tc.tile_pool(name="rd", bufs=2))
        e_p = ctx.enter_context(tc.tile_pool(name="ep", bufs=7))
        o_p = ctx.enter_context(tc.tile_pool(name="op", bufs=4))
        nr_p = ctx.enter_context(tc.tile_pool(name="nrp", bufs=3))
        at_p = ctx.enter_context(tc.tile_pool(name="atp", bufs=2))
        att_p = ctx.enter_context(tc.tile_pool(name="attp", bufs=2))

        ps_big = ctx.enter_context(tc.tile_pool(name="psb", bufs=6, space="PSUM"))
        ps_col = ctx.enter_context(tc.tile_pool(name="psc", bufs=2, space="PSUM"))

        # ---- persistent tiles ----
        wq_sb = {p: persist.tile([128, DP, 2, EL], FP8, tag=f"wq{p}",
                                 name=f"wq{p}_sb") for p in "hl"}
        wk_sb = {p: persist.tile([128, DP, 2, EL], FP8, tag=f"wk{p}",
                                 name=f"wk{p}_sb") for p in "hl"}
        wv_sb = {p: persist.tile([128, DP, 2, EL], FP8, tag=f"wv{p}",
                                 name=f"wv{p}_sb") for p in "hl"}
        wo_sb = {p: persist.tile([128, EP, 2, D], FP8, tag=f"wo{p}",
                                 name=f"wo{p}_sb") for p in "hl"}
        qT_sb = persist.tile([128, ET, 2 * CH], BF16, tag="qT")  # 2-chunk ring
        kT_sb = persist.tile([128, ET, S], BF16, tag="kT")
        v_sb = persist.tile([128, ST, EL], BF16, tag="v")
        msk_sb = persist.tile([128, 4, CH], BF16, tag="msk")
        id_sb = persist.tile([128, 128], F32, tag="ident")
        idb_sb = persist.tile([128, 128], BF16, tag="identb")
        wn_sb = persist.tile([128, 4], F32, tag="wn")
        ones_sb = persist.tile([128, 1], BF16, tag="ones")
        rk_sb = persist.tile([128, 2, ST], F32, tag="rk")   # f_k per (head, sk)
        nc.vector.memset(ones_sb, 1.0)

        # PE warm-up: keep the array busy through the initial DMA wait so the
        # p-state ramp completes before the first real projection matmuls.
        warm_sb = persist.tile([128, CH], BF16, tag="warm")
        nc.vector.memset(warm_sb, 0.0)

        def warm_mm():
            warm_mm.i += 1
            wp_ps = ps_col.tile([1, CH], F32, tag="col",
                                name=f"warm{warm_mm.i}")
            nc.tensor.matmul(wp_ps, ones_sb, warm_sb, start=True, stop=True)
        warm_mm.i = 0

        for _w in range(8):
            warm_mm()

        # split so the first projection's matmuls can start after 1/2 the load
        nc.sync.dma_start(
            out=wq_sb["h"][:, 0:DP // 2],
            in_=wq_d["h"][0:DP // 2].rearrange("t p i e -> p t i e"))
        nc.sync.dma_start(out=id_sb, in_=id_d)
        nc.sync.dma_start(out=idb_sb, in_=idb_d)
        nc.sync.dma_start(out=wn_sb, in_=wn_d)

        def load_wk():
            for p in "hl":
                nc.sync.dma_start(out=wk_sb[p],
                                  in_=wk_d[p].rearrange("t p i e -> p t i e"))

        def load_wv():
            for p in "hl":
                nc.sync.dma_start(out=wv_sb[p],
                                  in_=wv_d[p].rearrange("t p i e -> p t i e"))

        def load_msk():
            nc.sync.dma_start(out=msk_sb, in_=msk_d.rearrange("t p s -> p t s"))

        def load_wo():
            for p in "hl":
                nc.sync.dma_start(out=wo_sb[p],
                                  in_=wo_d[p].rearrange("t p i d -> p t i d"))

        # q/k are computed at 64x scale (weights host-prescaled); Sum(q^2)
        # is then 4096x, so the rmsnorm epsilon scales to match and the
        # rsqrt output absorbs the 1/64.
        LN_EPS = float(HD * EPS * WSCALE * WSCALE)

        def nr_rsqrt(dst, src_ps, n, scale16):
            """dst[128, n] = (src_ps + eps')^(-1/2) (*16), DVE only."""
            x = nr_p.tile([128, 8], F32, tag="nrx", name=f"nrx{nr_rsqrt.i}")[:, :n]
            nc.vector.tensor_scalar(out=x, in0=src_ps, scalar1=LN_EPS,
                                    scalar2=None, op0=mybir.AluOpType.add)
            s = nr_p.tile([128, 8], F32, tag="nry", name=f"nry{nr_rsqrt.i}")[:, :n]
            nc.vector.tensor_scalar(
                out=s.bitcast(I32), in0=x.bitcast(I32), scalar1=1,
                scalar2=None, op0=mybir.AluOpType.arith_shift_right)
            nc.vector.tensor_scalar(
                out=s.bitcast(I32), in0=s.bitcast(I32), scalar1=SQRT_MAGIC,
                scalar2=None, op0=mybir.AluOpType.add)
            for it in range(1):
                r = nr_p.tile([128, 8], F32, tag="nrt",
                              name=f"nrt{nr_rsqrt.i}_{it}")[:, :n]
                nc.vector.reciprocal(out=r, in_=s)
                nc.vector.tensor_mul(out=r, in0=r, in1=x)     # x / s
                nc.vector.tensor_add(out=s, in0=s, in1=r)
                nc.vector.tensor_scalar(out=s, in0=s, scalar1=0.5,
                                        scalar2=None, op0=mybir.AluOpType.mult)
            nr_rsqrt.i += 1
            nc.vector.reciprocal(out=dst, in_=s)
            if scale16:
                nc.vector.tensor_scalar(out=dst, in0=dst, scalar1=16.0,
                                        scalar2=None, op0=mybir.AluOpType.mult)

        nr_rsqrt.i = 0

        def col_to_row(src, n, eng=None):
            """[128, n] f32 cols -> [1, n*128] row on partition 0.

            row[0, 128*j + p] = src[p, j].  PE transpose + one copy + one
            SWDGE DMA — keeps the Pool engine's slow software-DGE path to a
            single transfer instead of n."""
            i = col_to_row.i
            col_to_row.i += 1
            dt = src.dtype
            ident = id_sb if dt == F32 else idb_sb
            tp_ps = ps_col.tile([8, 128], dt, tag="col", name=f"tp{i}")
            nc.tensor.matmul(tp_ps[0:n, :], src, ident, is_transpose=True)
            row8 = nr_p.tile([8, 128], dt, tag="row8", name=f"row8_{i}")
            nc.vector.tensor_copy(out=row8[0:n, :], in_=tp_ps[0:n, :])
            row = nr_p.tile([1, 1024], dt, tag="row", name=f"row{i}")
            (eng or nc.gpsimd).dma_start(
                out=row[0:1, 0:n * 128], in_=row8[0:n, :])
            return row
        col_to_row.i = 0

        def col_to_row_p0(src, n):
            """Like col_to_row but DMA-free: n per-column PE transposes into
            one partition-0 psum row + a single copy.  Slightly more PE, but
            the chain can't queue behind bulk DMA transfers."""
            i = col_to_row.i
            col_to_row.i += 1
            assert n <= 4
            row_ps = ps_col.tile([1, 512], F32, tag="col", name=f"rp{i}")
            for j in range(n):
                nc.tensor.matmul(row_ps[0:1, j * 128:(j + 1) * 128],
                                 src[:, j:j + 1], id_sb, is_transpose=True,
                                 skip_group_check=True)
            row = nr_p.tile([1, 1024], F32, tag="row", name=f"row{i}")
            nc.vector.tensor_copy(out=row[0:1, 0:n * 128],
                                  in_=row_ps[0:1, 0:n * 128])
            return row

        def _rope2(d1, d2, x1, x2, tab):
            """x1/x2 already carry the rmsnorm weight; tab = [cos, sin].
               d1 = x1*cos - x2*sin;  d2 = x2*cos + x1*sin"""
            t1 = rtmp_p.tile([128, CH], BF16, tag="rtmp")
            t2 = rtmp_p.tile([128, CH], BF16, tag="rtmp")
            nc.vector.tensor_mul(out=t1, in0=x1, in1=tab[:, 0, :])
            nc.vector.tensor_mul(out=t2, in0=x2, in1=tab[:, 1, :])
            nc.vector.tensor_sub(out=d1, in0=t1, in1=t2)
            t3 = rtmp_p.tile([128, CH], BF16, tag="rtmp")
            t4 = rtmp_p.tile([128, CH], BF16, tag="rtmp")
            nc.vector.tensor_mul(out=t3, in0=x2, in1=tab[:, 0, :])
            nc.vector.tensor_mul(out=t4, in0=x1, in1=tab[:, 1, :])
            nc.vector.tensor_add(out=d2, in0=t3, in1=t4)

        def _rope(dst_sb, et0, cs, x1, x2, tab):
            _rope2(dst_sb[:, et0, cs], dst_sb[:, et0 + 1, cs], x1, x2, tab)

        def phase_a(c, first=False):
            """QKV projections + rmsnorm + rope for chunk c."""
            cs = slice(c * CH, (c + 1) * CH)
            xh4 = []
            xl4 = []
            for T in range(DP // 2):
                th = xs_p.tile([128, 2, 2, CH], FP8, tag="xs")
                nc.sync.dma_start(out=th, in_=xh_d[T, :, c, :, :, :])
                xh4.append(th)
            if first:
                nc.sync.dma_start(
                    out=wq_sb["h"][:, DP // 2:],
                    in_=wq_d["h"][DP // 2:].rearrange("t p i e -> p t i e"))
                nc.sync.dma_start(
                    out=wq_sb["l"],
                    in_=wq_d["l"].rearrange("t p i e -> p t i e"))
            for T in range(DP // 2):
                tl = xs_p.tile([128, 2, 2, CH], FP8, tag="xs")
                nc.sync.dma_start(out=tl, in_=xl_d[T, :, c, :, :, :])
                xl4.append(tl)
            xh = [xh4[t // 2][:, t % 2] for t in range(DP)]
            xl = [xl4[t // 2][:, t % 2] for t in range(DP)]
            rt_t = rt_p.tile([128, 2, CH], BF16, tag="rt")
            nc.sync.dma_start(out=rt_t,
                              in_=rt_d[:, :, cs].rearrange("t p s -> p t s"))
            if first:
                load_wk()

            def proj_qk(ps, w_sb, et, warm=False):
                """ps[128, CH] += x @ W[:, et-cols]: 3-term fp8 DR pairs,
                term-major so the first 8 matmuls need only hi planes.
                warm: interleave no-op matmuls so the PE stream has filler
                while the very first x/weight loads stream in."""
                n = 3 * DP
                i = 0
                for (wp, xs) in ((w_sb["h"], xh), (w_sb["l"], xh),
                                 (w_sb["h"], xl)):
                    for t in range(DP):
                        nc.tensor.matmul(
                            ps, wp[:, t, :, et * 128:(et + 1) * 128], xs[t],
                            start=(i == 0), stop=(i == n - 1), perf_mode=DR)
                        if warm:
                            warm_mm()
                        i += 1

            # ---------- q ----------
            rq_ps = ps_col.tile([128, 8], F32, tag="col")
            qr = []
            sqs = []
            for et in range(ET):
                q_ps = ps_big.tile([128, CH], F32, tag="big")
                proj_qk(q_ps, wq_sb, et)
                t = qr_p.tile([128, CH], BF16, tag="qr")
                # copy folds in the per-dim rmsnorm weight (w1/w2 by parity)
                nc.scalar.activation(out=t, in_=q_ps,
                                     func=mybir.ActivationFunctionType.Copy,
                                     bias=0.0,
                                     scale=wn_sb[:, et % 2:et % 2 + 1])
                qr.append(t)
                sqt = sq_p.tile([128, CH], BF16, tag="sq")
                nc.scalar.activation(out=sqt, in_=q_ps,
                                     func=mybir.ActivationFunctionType.Square,
                                     bias=0.0, scale=1.0)
                sqs.append(sqt)
            def q_colmms():
                for et in range(ET):
                    hh = et // 2
                    for j in range(4):
                        nc.tensor.matmul(
                            rq_ps[:, 4 * hh + j: 4 * hh + j + 1],
                            sqs[et][:, j * 128:(j + 1) * 128], ones_sb,
                            start=(et == 0 and j == 0),
                            stop=(et == ET - 1 and j == 3))
            q_colmms()

            qcs = slice((c % 2) * CH, (c % 2 + 1) * CH)   # qT ring slot

            def q_tail():
                # rotate first (only needs qr + tables), scale by the
                # broadcast rsqrt last — so the rope math never waits on the
                # NR/transpose/broadcast chain.  The scale path runs in bf16.
                us = []
                for hh in range(2):
                    u1 = qs_p.tile([128, CH], BF16, tag="qs",
                                   name=f"u1_{c}_{hh}")
                    u2 = qs_p.tile([128, CH], BF16, tag="qs",
                                   name=f"u2_{c}_{hh}")
                    _rope2(u1, u2, qr[2 * hh], qr[2 * hh + 1], rt_t)
                    us.append((u1, u2))
                rq_sb = nr_p.tile([128, 8], F32, tag="rq")
                nr_rsqrt(rq_sb, rq_ps, 8, scale16=False)
                rq_bf = nr_p.tile([128, 8], BF16, tag="rqb")
                nc.vector.tensor_copy(out=rq_bf, in_=rq_sb)
                t_row = col_to_row(rq_bf, 8)
                for hh in range(2):
                    rbt = rb_p.tile([128, CH], BF16, tag="rb")
                    nc.gpsimd.partition_broadcast(
                        rbt, t_row[0:1, hh * CH:(hh + 1) * CH])
                    nc.vector.tensor_mul(out=qT_sb[:, 2 * hh, qcs],
                                         in0=us[hh][0], in1=rbt)
                    nc.vector.tensor_mul(out=qT_sb[:, 2 * hh + 1, qcs],
                                         in0=us[hh][1], in1=rbt)
            with tc.high_priority():
                q_tail()   # latency-critical: scores of this chunk need qT
            if first:
                load_wv()

            # ---------- k ----------
            rk_ps = ps_col.tile([128, 8], F32, tag="col")
            kr = []
            ksqs = []
            for et in range(ET):
                k_ps = ps_big.tile([128, CH], F32, tag="big")
                proj_qk(k_ps, wk_sb, et)
                t = qr_p.tile([128, CH], BF16, tag="qr")
                nc.scalar.activation(out=t, in_=k_ps,
                                     func=mybir.ActivationFunctionType.Copy,
                                     bias=0.0,
                                     scale=wn_sb[:, 2 + et % 2:3 + et % 2])
                kr.append(t)
                sqt = sq_p.tile([128, CH], BF16, tag="sq")
                nc.scalar.activation(out=sqt, in_=k_ps,
                                     func=mybir.ActivationFunctionType.Square,
                                     bias=0.0, scale=1.0)
                ksqs.append(sqt)
            def k_colmms():
                for et in range(ET):
                    hh = et // 2
                    for j in range(4):
                        nc.tensor.matmul(
                            rk_ps[:, 4 * hh + j: 4 * hh + j + 1],
                            ksqs[et][:, j * 128:(j + 1) * 128], ones_sb,
                            start=(et == 0 and j == 0),
                            stop=(et == ET - 1 and j == 3))
            k_colmms()

            def k_tail():
                nr_rsqrt(rk_sb[:, :, 4 * c:4 * c + 4], rk_ps, 8,
                         scale16=True)
                for hh in range(2):
                    _rope(kT_sb, 2 * hh, cs, kr[2 * hh], kr[2 * hh + 1],
                          rt_t)
            with tc.high_priority():
                k_tail()   # latency-critical: scores need kT + rk scale

            # ---------- v ----------
            for st in range(4):
                v_ps = ps_big.tile([128, EL], F32, tag="big")
                n = 3 * DP
                i = 0
                for (x4s, wp) in ((xh4, wv_sb["h"]), (xl4, wv_sb["h"]),
                                  (xh4, wv_sb["l"])):
                    for t in range(DP):
                        T, tt = divmod(t, 2)
                        nc.tensor.matmul(
                            v_ps, x4s[T][:, tt, :, st * 128:(st + 1) * 128],
                            wp[:, t, :, :],
                            start=(i == 0), stop=(i == n - 1), perf_mode=DR)
                        i += 1
                # copy undoes the 64x weight prescale
                nc.scalar.activation(out=v_sb[:, 4 * c + st, :], in_=v_ps,
                                     func=mybir.ActivationFunctionType.Copy,
                                     bias=0.0, scale=1.0 / WSCALE)
            if first:
                load_msk()
            return lambda: None

        def phase_b(c, hh, aTh_t, aTl_t, filler=None):
            """Attention for (chunk c, head hh), software-pipelined: av/den
            matmuls for tile k are emitted after the scores matmuls for tile
            k+1 so the PE never waits in-order on exp(k)."""
            cs = slice(c * CH, (c + 1) * CH)
            n_sk = 4 * c + 4
            av_ps = {i: ps_big.tile([128, CH], F32, tag="big",
                                    name=f"av{c}_{hh}_{i}") for i in range(2)}
            den_ps = ps_col.tile([128, 4], F32, tag="col",
                                 name=f"den{c}_{hh}")

            def consume(e_t, skt, lo):
                first, last = (skt == 0), (skt == n_sk - 1)
                for half in range(2):
                    nc.tensor.matmul(
                        av_ps[half][:, lo:],
                        v_sb[:, skt, hh * 256 + half * 128:
                             hh * 256 + (half + 1) * 128],
                        e_t[:, lo:], start=first, stop=last)
                for j in range(lo // 128, 4):
                    nc.tensor.matmul(
                        den_ps[:, j:j + 1],
                        e_t[:, j * 128:(j + 1) * 128], ones_sb,
                        start=(first and j == 0), stop=(last and j == 3))

            from collections import deque
            pend = deque()
            for skt in range(n_sk):
                rel = skt - 4 * c
                # columns below 128*rel of a diagonal tile are fully masked
                lo = max(rel, 0) * 128
                sc_ps = ps_big.tile([128, CH], F32, tag="big",
                                    name=f"sc{c}_{hh}_{skt}")
                for half in range(2):
                    et = 2 * hh + half
                    nc.tensor.matmul(
                        sc_ps[:, lo:],
                        kT_sb[:, et, skt * 128:(skt + 1) * 128],
                        qT_sb[:, et, (c % 2) * CH + lo:(c % 2 + 1) * CH],
                        start=(half == 0), stop=(half == 1))
                e_t = e_p.tile([128, CH], BF16, tag="ep",
                               name=f"et{c}_{hh}_{skt}")
                nc.scalar.activation(out=e_t[:, lo:], in_=sc_ps[:, lo:],
                                     func=mybir.ActivationFunctionType.Exp,
                                     bias=0.0,
                                     scale=rk_sb[:, hh, skt:skt + 1])
                if rel >= 0:
                    nc.vector.tensor_mul(out=e_t[:, lo:], in0=e_t[:, lo:],
                                         in1=msk_sb[:, rel, lo:])
                if len(pend) >= 3:
                    consume(*pend.popleft())
                if filler is not None:
                    for fn in next(filler, []) or []:
                        fn()
                pend.append((e_t, skt, lo))
            while pend:
                consume(*pend.popleft())
            # copy av out of PSUM immediately so the psb ring slot frees fast
            avs = []
            for half in range(2):
                t = qs_p.tile([128, CH], BF16, tag="qs",
                              name=f"avs{c}_{hh}_{half}")
                nc.scalar.copy(out=t, in_=av_ps[half])
                avs.append(t)
            with tc.high_priority():
                dinv = nr_p.tile([128, 4], F32, tag="dinv",
                                 name=f"dinv{c}_{hh}")
                nc.vector.reciprocal(out=dinv, in_=den_ps)
                dinv_bf = nr_p.tile([128, 4], BF16, tag="dinvb",
                                    name=f"dinvb{c}_{hh}")
                nc.vector.tensor_copy(out=dinv_bf, in_=dinv)
                t_row = col_to_row(dinv_bf, 4)
                rd_t = rd_p.tile([128, CH], BF16, tag="rd", name=f"rd{c}_{hh}")
                nc.gpsimd.partition_broadcast(rd_t, t_row[0:1, 0:CH])
                for half in range(2):
                    et = 2 * hh + half
                    t = att_p.tile([128, CH], BF16, tag="att",
                                   name=f"att{c}_{hh}_{half}")
                    nc.vector.tensor_mul(out=t, in0=avs[half], in1=rd_t)
                    nc.scalar.copy(out=aTh_t[:, et, :], in_=t)
                    nc.vector.tensor_sub(out=aTl_t[:, et, :], in0=t,
                                         in1=aTh_t[:, et, :])

        def c_burst(c, aTh_t, aTl_t, dp):
            """Wo product for dout pair (2*dp, 2*dp+1), one bf16 store."""
            cs = slice(c * CH, (c + 1) * CH)
            o_t = o_p.tile([128, 2, CH], BF16, tag="op", name=f"ot{c}_{dp}")
            for j in range(2):
                dout = 2 * dp + j
                o_ps = ps_big.tile([128, CH], F32, tag="big",
                                   name=f"ops{c}_{dout}")
                i = 0
                for t in range(EP):
                    for (wp, ap) in ((wo_sb["h"], aTh_t), (wo_sb["l"], aTh_t),
                                     (wo_sb["h"], aTl_t)):
                        nc.tensor.matmul(
                            o_ps, wp[:, t, :, dout * 128:(dout + 1) * 128],
                            ap[:, 2 * t:2 * t + 2, :],
                            start=(i == 0), stop=(i == 3 * EP - 1),
                            perf_mode=DR)
                        i += 1
                if j == 0:
                    nc.vector.tensor_copy(out=o_t[:, j, :], in_=o_ps)
                else:
                    nc.scalar.copy(out=o_t[:, j, :], in_=o_ps)
            if c == 3:
                # last chunk: split dispatch between the HWDGE queues and the
                # Pool SWDGE queue so the drain isn't HWDGE-serialized; keep
                # the final bursts on the fast HWDGE queues
                eng = nc.gpsimd if dp < 3 else (
                    nc.sync if dp % 2 == 0 else nc.scalar)
            else:
                eng = nc.sync if dp % 2 == 0 else nc.scalar
            eng.dma_start(
                out=out_d[dp * 256:(dp + 1) * 256, cs].rearrange(
                    "(j p) s -> p j s", p=128),
                in_=o_t)

        def phase_c(c, aTh_t, aTl_t):
            for dp in range(DT // 2):
                c_burst(c, aTh_t, aTl_t, dp)

        def c_filler(c, aTh_t, aTl_t, lo, hi):
            for dp in range(lo, hi):
                yield [lambda c=c, ah=aTh_t, al=aTl_t, d=dp:
                       c_burst(c, ah, al, d)]

        # ---- schedule ----
        aTh = {}
        aTl = {}

        def new_aT(c):
            aTh[c] = at_p.tile([128, ET, CH], FP8, tag="ath", name=f"aTh{c}")
            aTl[c] = at_p.tile([128, ET, CH], FP8, tag="atl", name=f"aTl{c}")

        kt0 = phase_a(0, first=True)
        kt0()
        new_aT(0)
        phase_b(0, 0, aTh[0], aTl[0])
        kt1 = phase_a(1)
        kt1()
        phase_b(0, 1, aTh[0], aTl[0])
        load_wo()   # needed first by the chunk-0 Wo fillers in phase_b(1,*)
        def _delayed(gen, skip):
            for _ in range(skip):
                yield []
            yield from gen
        kt2 = phase_a(2)
        new_aT(1)
        phase_b(1, 0, aTh[1], aTl[1],
                filler=_delayed(c_filler(0, aTh[0], aTl[0], 0, 4), 3))
        phase_b(1, 1, aTh[1], aTl[1], filler=c_filler(0, aTh[0], aTl[0], 4, 8))
        kt2()
        kt3 = phase_a(3)
        new_aT(2)
        phase_b(2, 0, aTh[2], aTl[2],
                filler=_delayed(c_filler(1, aTh[1], aTl[1], 0, 4), 3))
        phase_b(2, 1, aTh[2], aTl[2], filler=c_filler(1, aTh[1], aTl[1], 4, 8))
        kt3()
        new_aT(3)
        phase_b(3, 0, aTh[3], aTl[3],
                filler=_delayed(c_filler(2, aTh[2], aTl[2], 0, 4), 8))
        phase_b(3, 1, aTh[3], aTl[3])
        for _d in range(4, 8):
            c_burst(2, aTh[2], aTl[2], _d)
        phase_c(3, aTh[3], aTl[3])


def _host_tables(position_ids, q_norm_w, k_norm_w):
    pos = np.asarray(position_ids).astype(np.float64)
    inv = 1.0 / (10000.0 ** (np.arange(0, HD, 2, dtype=np.float64) / HD))
    ang = pos[:, None] * inv[None, :]                      # [S, 128]
    rt = np.stack([np.cos(ang).T, np.sin(ang).T]).astype(NBF)  # [2, 128, S]
    qw = np.asarray(q_norm_w, np.float32)
    kw = np.asarray(k_norm_w, np.float32)
    wn = np.stack([qw[:HALF], qw[HALF:], kw[:HALF], kw[HALF:]],
                  axis=1).astype(np.float32)               # [128, 4]
    return rt, wn


def _host_masks():
    p = np.arange(HALF)[:, None]
    n = np.arange(CH)[None, :]
    return np.stack(
        [(rel * HALF + p <= n) for rel in range(4)]).astype(NBF)


def _pack_pair(mT):
    """[K, N] f32 -> (hi, lo) fp8 planes [K/256, 128, 2, N] for DoubleRow.

    plane[t, p, i, n] = m[256*t + 128*i + p, n]"""
    hi = mT.astype(NF8)
    lo = (mT - hi.astype(np.float32)).astype(NF8)

    def pk(a):
        k, n = a.shape
        return np.ascontiguousarray(
            a.reshape(k // 256, 2, 128, n).transpose(0, 2, 1, 3))
    return pk(hi), pk(lo)


def _pack_x(xT):
    """[D, S] f32 -> (hi, lo) fp8 [T, p, c, tt, i, s'] with
    element = xT[512*T + 256*tt + 128*i + p, c*CH + s'] (2KB contiguous
    per (T, p, c) line)."""
    hi = xT.astype(NF8)
    lo = (xT - hi.astype(np.float32)).astype(NF8)

    def pk(a):
        return np.ascontiguousarray(
            a.reshape(DP // 2, 2, 2, 128, NCH, CH).transpose(0, 3, 4, 1, 2, 5))
    return pk(hi), pk(lo)


def kernel(**inputs):
    x = np.asarray(inputs["x"], np.float32)
    sc = np.float32(WSCALE)
    Wq = np.asarray(inputs["Wq"], np.float32) * sc
    Wk = np.asarray(inputs["Wk"], np.float32) * sc
    Wv = np.asarray(inputs["Wv"], np.float32) * sc
    Wo = np.asarray(inputs["Wo"], np.float32) * sc
    rt, wn = _host_tables(inputs["position_ids"],
                          inputs["q_norm_w"], inputs["k_norm_w"])
    masks = _host_masks()

    if "nc" not in _CACHE:
        _CACHE["nc"] = _build()
    nc = _CACHE["nc"]
    ident = np.eye(128, dtype=np.float32)
    identb = np.eye(128, dtype=NBF)

    xpk = [_pack_x(np.ascontiguousarray(x[b].T)) for b in range(B)]
    in_maps = []
    for core in range(N_CORES):
        b, g = divmod(core, 4)
        es = slice(g * EL, (g + 1) * EL)
        wqh, wql = _pack_pair(np.ascontiguousarray(Wq[es, :].T))
        wkh, wkl = _pack_pair(np.ascontiguousarray(Wk[es, :].T))
        wvh, wvl = _pack_pair(np.ascontiguousarray(Wv[es, :].T))
        woh, wol = _pack_pair(np.ascontiguousarray(Wo[:, es].T))
        in_maps.append({
            "xh": xpk[b][0], "xl": xpk[b][1],
            "wqh": wqh, "wql": wql,
            "wkh": wkh, "wkl": wkl,
            "wvh": wvh, "wvl": wvl,
            "woh": woh, "wol": wol,
            "ident": ident, "identb": identb, "rt": rt, "wn": wn,
            "masks": masks,
        })
    res = run_bass_kernel_spmd(nc, in_maps, core_ids=list(range(N_CORES)))
    out = np.zeros((B, S, D), np.float32)
    inv = np.float32(1.0 / WSCALE)
    for core in range(N_CORES):
        b = core // 4
        out[b] += res.results[core]["outT"].astype(np.float32).T * inv
    return out


# revision 152
# speedup vs baseline: 1.0001x; 1.0001x over previous
"""Multi-head causal attention (RoPE + QK-RMSNorm) on 8 TRN2 NeuronCores.

Sharding: data parallel on batch (2) x tensor parallel on heads (4 groups of
2 heads).  core = 4*b + g computes, for batch b, heads [2g, 2g+1]:
  q/k/v projections (E-sliced), qk-rmsnorm + rope, causal attention, and the
  Wo partial product over its E slice.  Host sums the 4 partials per batch.

Everything on device runs in the "transposed" orientation:
  qT/kT [e, s], v [s, e], scoresT [sk, sq], out_T [d_out, s]
so no on-chip data transposes are needed; softmax denominators and rms sums
are computed with ones-matmuls on the TensorEngine in *column* layout
[128 positions, few cols].  1/x uses the DVE divide unit; 1/sqrt(x) uses a
sqrt bit-hack seed + 2 Babylonian iterations + reciprocal, all on DVE, so
the ScalarEngine only ever needs the exp table set (no table thrashing).

fp8 DoubleRow: the projection (x @ Wq/Wk/Wv) and output (aT @ Wo) matmuls
run as fp8e4 DoubleRow pairs over 256-deep contractions.  Each operand is
represented as hi+lo fp8 planes (lo = residual), and the product keeps the
3 significant cross terms (hh, lh, hl), dropping lo*lo: 3 DR matmuls per
256-contraction = 1.5 cycles/col vs 2.0 for bf16 (25% PE savings) at
slightly *better* than bf16 accuracy.  Weights are host-prescaled by 64 to
dodge fp8 subnormals; the scale is undone via RMSNorm for q/k, via the
PSUM->SBUF copy scale for v, and on the host for Wo.  The attention matmuls
(scores, av) stay bf16: 2-terming q/k/e on-chip costs more DVE than it
saves on PE.

Latency lessons baked in (cost model: all DMA transfers serialize through
one DMA_ENGINES device at ~360 GB/s, each DMA also holds the shared HWDGE
~625ns; per-column gpsimd DMAs cost ~1us each on the SWDGE path):
 - col->row transposes go through the PE (permutation matmul vs a host
   identity) + one copy + one SWDGE DMA, not n per-column DMAs.
 - the rmsnorm weights fold into the q/k PSUM-copy scale (per-partition
   scale AP), so the rope tables are pure cos/sin (1MB not 4MB of DMA).
 - q-rope is factored: rotate first (needs only qr), multiply by the
   broadcast rsqrt last, so rope math never waits on the NR chain; the
   rsqrt/denominator broadcast paths run in bf16 (2x DVE).
 - q/k tails are emitted inline (overlap the k/v-section matmuls) at high
   scheduler priority; rsqrt uses 1 Babylonian iteration (0.06% scale err).
 - bulk loads are staggered (wq split, wo deferred past phase_a(1)) so
   latency-critical small DMAs do not queue behind megabytes of weights.
 - av is copied out of PSUM immediately (psb ring slots never wait on the
   denominator chain); x is packed [T,p,c,tt,i,s'] so every load line is
   2KB contiguous; outputs store as bf16, 2 douts per DMA, and the last
   chunk splits stores across HWDGE/SWDGE dispatch paths.
 - a few warm-up matmuls cover the initial DMA wait so the PE p-state ramp
   is done before real work.

Scheduling: the attention inner loop is software-pipelined depth-3 (scores
k+3 issue before av/den k), diagonal tiles compute only their live columns,
rms col-matmuls are hoisted behind each projection section, Wo output
bursts are injected as PE filler into the later attention groups (delayed
3 slots so they never stall on the previous chunk's aT hi/lo chain).

TimelineSim: 242005 ns/core (baseline 308002, -21.4%), rms rel err 8.7e-3.
"""

import math

import numpy as np
import ml_dtypes

import concourse.bass as bass
import concourse.tile as tile
from concourse import bacc, mybir
from concourse.bass_utils import run_bass_kernel_spmd

# Problem shapes (hardcoded per instructions).
B = 2
S = 2048
D = 2048
H = 8
HD = 256
HALF = 128
EL = 512          # E columns per core (2 heads)
CH = 512          # sq chunk size
NCH = S // CH     # 4
DT = D // 128     # 16 k-tiles over D
DP = DT // 2      # 8 k-tile *pairs* over D (DoubleRow)
ET = EL // 128    # 4 e-tiles
EP = ET // 2      # 2 e-tile pairs (DoubleRow Wo contraction)
ST = S // 128     # 16 s-tiles
EPS = 1e-6
N_CORES = 8
WSCALE = 64.0     # host pre-scale on Wq/Wk/Wv/Wo to avoid fp8 subnormals

BF16 = mybir.dt.bfloat16
F32 = mybir.dt.float32
FP8 = mybir.dt.float8e4
I32 = mybir.dt.int32
NBF = ml_dtypes.bfloat16
NF8 = ml_dtypes.float8_e4m3
DR = mybir.MatmulPerfMode.DoubleRow

SQRT_MAGIC = 0x1FBD1DF5     # sqrt(x) seed: (bits(x) >> 1) + MAGIC

_CACHE: dict = {}


def _build(reps: int = 1):
    nc = bacc.Bacc("TRN2", target_bir_lowering=False, debug=False,
                   num_devices=N_CORES)

    # x pairs-of-pairs: [T, p, c, tt, i, s'] = xT[512T+256tt+128i+p, c*CH+s']
    # so each (T, p, c) line is 2KB contiguous in DRAM.
    xh_d = nc.dram_tensor("xh", [DP // 2, 128, NCH, 2, 2, CH], FP8,
                          kind="ExternalInput").ap()
    xl_d = nc.dram_tensor("xl", [DP // 2, 128, NCH, 2, 2, CH], FP8,
                          kind="ExternalInput").ap()
    wq_d = {p: nc.dram_tensor(f"wq{p}", [DP, 128, 2, EL], FP8,
                              kind="ExternalInput").ap() for p in "hl"}
    wk_d = {p: nc.dram_tensor(f"wk{p}", [DP, 128, 2, EL], FP8,
                              kind="ExternalInput").ap() for p in "hl"}
    wv_d = {p: nc.dram_tensor(f"wv{p}", [DP, 128, 2, EL], FP8,
                              kind="ExternalInput").ap() for p in "hl"}
    wo_d = {p: nc.dram_tensor(f"wo{p}", [EP, 128, 2, D], FP8,
                              kind="ExternalInput").ap() for p in "hl"}
    id_d = nc.dram_tensor("ident", [128, 128], F32, kind="ExternalInput").ap()
    idb_d = nc.dram_tensor("identb", [128, 128], BF16, kind="ExternalInput").ap()
    rt_d = nc.dram_tensor("rt", [2, HALF, S], BF16, kind="ExternalInput").ap()
    wn_d = nc.dram_tensor("wn", [128, 4], F32, kind="ExternalInput").ap()
    msk_d = nc.dram_tensor("masks", [4, HALF, CH], BF16, kind="ExternalInput").ap()
    out_d = nc.dram_tensor("outT", [D, S], BF16, kind="ExternalOutput").ap()

    with tile.TileContext(nc) as tc:
        for _ in range(reps):
            _emit(tc, nc, xh_d, xl_d, wq_d, wk_d, wv_d, wo_d, id_d, idb_d,
                  rt_d, wn_d, msk_d, out_d)
    nc.compile()
    return nc


def _emit(tc, nc, xh_d, xl_d, wq_d, wk_d, wv_d, wo_d, id_d, idb_d, rt_d,
          wn_d, msk_d, out_d):
    from contextlib import ExitStack
    ctx = ExitStack()
    with ctx:
        persist = ctx.enter_context(tc.tile_pool(name="persist", bufs=1))
        xs_p = ctx.enter_context(tc.tile_pool(name="xs", bufs=16))
        rt_p = ctx.enter_context(tc.tile_pool(name="rt", bufs=2))
        sq_p = ctx.enter_context(tc.tile_pool(name="sq", bufs=4))
        qr_p = ctx.enter_context(tc.tile_pool(name="qr", bufs=10))
        qs_p = ctx.enter_context(tc.tile_pool(name="qs", bufs=6))
        rtmp_p = ctx.enter_context(tc.tile_pool(name="rtmp", bufs=4))
        rb_p = ctx.enter_context(tc.tile_pool(name="rb", bufs=2))
        rd_p = ctx.enter_context(tc.tile_pool(name="rd", bufs=2))
        e_p = ctx.enter_context(tc.tile_pool(name="ep", bufs=8))
        o_p = ctx.enter_context(tc.tile_pool(name="op", bufs=4))
        nr_p = ctx.enter_context(tc.tile_pool(name="nrp", bufs=3))
        at_p = ctx.enter_context(tc.tile_pool(name="atp", bufs=2))
        att_p = ctx.enter_context(tc.tile_pool(name="attp", bufs=2))

        ps_big = ctx.enter_context(tc.tile_pool(name="psb", bufs=6, space="PSUM"))
        ps_col = ctx.enter_context(tc.tile_pool(name="psc", bufs=2, space="PSUM"))

        # ---- persistent tiles ----
        wq_sb = {p: persist.tile([128, DP, 2, EL], FP8, tag=f"wq{p}",
                                 name=f"wq{p}_sb") for p in "hl"}
        wk_sb = {p: persist.tile([128, DP, 2, EL], FP8, tag=f"wk{p}",
                                 name=f"wk{p}_sb") for p in "hl"}
        wv_sb = {p: persist.tile([128, DP, 2, EL], FP8, tag=f"wv{p}",
                                 name=f"wv{p}_sb") for p in "hl"}
        wo_sb = {p: persist.tile([128, EP, 2, D], FP8, tag=f"wo{p}",
                                 name=f"wo{p}_sb") for p in "hl"}
        qT_sb = persist.tile([128, ET, 2 * CH], BF16, tag="qT")  # 2-chunk ring
        kT_sb = persist.tile([128, ET, S], BF16, tag="kT")
        v_sb = persist.tile([128, ST, EL], BF16, tag="v")
        msk_sb = persist.tile([128, 4, CH], BF16, tag="msk")
        id_sb = persist.tile([128, 128], F32, tag="ident")
        idb_sb = persist.tile([128, 128], BF16, tag="identb")
        wn_sb = persist.tile([128, 4], F32, tag="wn")
        ones_sb = persist.tile([128, 1], BF16, tag="ones")
        rk_sb = persist.tile([128, 2, ST], F32, tag="rk")   # f_k per (head, sk)
        nc.vector.memset(ones_sb, 1.0)

        # PE warm-up: keep the array busy through the initial DMA wait so the
        # p-state ramp completes before the first real projection matmuls.
        warm_sb = persist.tile([128, CH], BF16, tag="warm")
        nc.vector.memset(warm_sb, 0.0)

        def warm_mm():
            warm_mm.i += 1
            wp_ps = ps_col.tile([1, CH], F32, tag="col",
                                name=f"warm{warm_mm.i}")
            nc.tensor.matmul(wp_ps, ones_sb, warm_sb, start=True, stop=True)
        warm_mm.i = 0

        for _w in range(8):
            warm_mm()

        # split so the first projection's matmuls can start after 1/2 the load
        nc.sync.dma_start(
            out=wq_sb["h"][:, 0:DP // 2],
            in_=wq_d["h"][0:DP // 2].rearrange("t p i e -> p t i e"))
        nc.sync.dma_start(out=id_sb, in_=id_d)
        nc.sync.dma_start(out=idb_sb, in_=idb_d)
        nc.sync.dma_start(out=wn_sb, in_=wn_d)

        def load_wk():
            for p in "hl":
                nc.sync.dma_start(out=wk_sb[p],
                                  in_=wk_d[p].rearrange("t p i e -> p t i e"))

        def load_wv():
            for p in "hl":
                nc.sync.dma_start(out=wv_sb[p],
                                  in_=wv_d[p].rearrange("t p i e -> p t i e"))

        def load_msk():
            nc.sync.dma_start(out=msk_sb, in_=msk_d.rearrange("t p s -> p t s"))

        def load_wo():
            for p in "hl":
                nc.sync.dma_start(out=wo_sb[p],
                                  in_=wo_d[p].rearrange("t p i d -> p t i d"))

        # q/k are computed at 64x scale (weights host-prescaled); Sum(q^2)
        # is then 4096x, so the rmsnorm epsilon scales to match and the
        # rsqrt output absorbs the 1/64.
        LN_EPS = float(HD * EPS * WSCALE * WSCALE)

        def nr_rsqrt(dst, src_ps, n, scale16):
            """dst[128, n] = (src_ps + eps')^(-1/2) (*16), DVE only."""
            x = nr_p.tile([128, 8], F32, tag="nrx", name=f"nrx{nr_rsqrt.i}")[:, :n]
            nc.vector.tensor_scalar(out=x, in0=src_ps, scalar1=LN_EPS,
                                    scalar2=None, op0=mybir.AluOpType.add)
            s = nr_p.tile([128, 8], F32, tag="nry", name=f"nry{nr_rsqrt.i}")[:, :n]
            nc.vector.tensor_scalar(
                out=s.bitcast(I32), in0=x.bitcast(I32), scalar1=1,
                scalar2=None, op0=mybir.AluOpType.arith_shift_right)
            nc.vector.tensor_scalar(
                out=s.bitcast(I32), in0=s.bitcast(I32), scalar1=SQRT_MAGIC,
                scalar2=None, op0=mybir.AluOpType.add)
            for it in range(1):
                r = nr_p.tile([128, 8], F32, tag="nrt",
                              name=f"nrt{nr_rsqrt.i}_{it}")[:, :n]
                nc.vector.reciprocal(out=r, in_=s)
                nc.vector.tensor_mul(out=r, in0=r, in1=x)     # x / s
                nc.vector.tensor_add(out=s, in0=s, in1=r)
                nc.vector.tensor_scalar(out=s, in0=s, scalar1=0.5,
                                        scalar2=None, op0=mybir.AluOpType.mult)
            nr_rsqrt.i += 1
            nc.vector.reciprocal(out=dst, in_=s)
            if scale16:
                nc.vector.tensor_scalar(out=dst, in0=dst, scalar1=16.0,
                                        scalar2=None, op0=mybir.AluOpType.mult)

        nr_rsqrt.i = 0

        def col_to_row(src, n, eng=None):
            """[128, n] f32 cols -> [1, n*128] row on partition 0.

            row[0, 128*j + p] = src[p, j].  PE transpose + one copy + one
            SWDGE DMA — keeps the Pool engine's slow software-DGE path to a
            single transfer instead of n."""
            i = col_to_row.i
            col_to_row.i += 1
            dt = src.dtype
            ident = id_sb if dt == F32 else idb_sb
            tp_ps = ps_col.tile([8, 128], dt, tag="col", name=f"tp{i}")
            nc.tensor.matmul(tp_ps[0:n, :], src, ident, is_transpose=True)
            row8 = nr_p.tile([8, 128], dt, tag="row8", name=f"row8_{i}")
            nc.vector.tensor_copy(out=row8[0:n, :], in_=tp_ps[0:n, :])
            row = nr_p.tile([1, 1024], dt, tag="row", name=f"row{i}")
            (eng or nc.gpsimd).dma_start(
                out=row[0:1, 0:n * 128], in_=row8[0:n, :])
            return row
        col_to_row.i = 0

        def col_to_row_p0(src, n):
            """Like col_to_row but DMA-free: n per-column PE transposes into
            one partition-0 psum row + a single copy.  Slightly more PE, but
            the chain can't queue behind bulk DMA transfers."""
            i = col_to_row.i
            col_to_row.i += 1
            assert n <= 4
            row_ps = ps_col.tile([1, 512], F32, tag="col", name=f"rp{i}")
            for j in range(n):
                nc.tensor.matmul(row_ps[0:1, j * 128:(j + 1) * 128],
                                 src[:, j:j + 1], id_sb, is_transpose=True,
                                 skip_group_check=True)
            row = nr_p.tile([1, 1024], F32, tag="row", name=f"row{i}")
            nc.vector.tensor_copy(out=row[0:1, 0:n * 128],
                                  in_=row_ps[0:1, 0:n * 128])
            return row

        def _rope2(d1, d2, x1, x2, tab):
            """x1/x2 already carry the rmsnorm weight; tab = [cos, sin].
               d1 = x1*cos - x2*sin;  d2 = x2*cos + x1*sin"""
            t1 = rtmp_p.tile([128, CH], BF16, tag="rtmp")
            t2 = rtmp_p.tile([128, CH], BF16, tag="rtmp")
            nc.vector.tensor_mul(out=t1, in0=x1, in1=tab[:, 0, :])
            nc.vector.tensor_mul(out=t2, in0=x2, in1=tab[:, 1, :])
            nc.vector.tensor_sub(out=d1, in0=t1, in1=t2)
            t3 = rtmp_p.tile([128, CH], BF16, tag="rtmp")
            t4 = rtmp_p.tile([128, CH], BF16, tag="rtmp")
            nc.vector.tensor_mul(out=t3, in0=x2, in1=tab[:, 0, :])
            nc.vector.tensor_mul(out=t4, in0=x1, in1=tab[:, 1, :])
            nc.vector.tensor_add(out=d2, in0=t3, in1=t4)

        def _rope(dst_sb, et0, cs, x1, x2, tab):
            _rope2(dst_sb[:, et0, cs], dst_sb[:, et0 + 1, cs], x1, x2, tab)

        def phase_a(c, first=False):
            """QKV projections + rmsnorm + rope for chunk c."""
            cs = slice(c * CH, (c + 1) * CH)
            xh4 = []
            xl4 = []
            for T in range(DP // 2):
                th = xs_p.tile([128, 2, 2, CH], FP8, tag="xs")
                nc.sync.dma_start(out=th, in_=xh_d[T, :, c, :, :, :])
                xh4.append(th)
            if first:
                nc.sync.dma_start(
                    out=wq_sb["h"][:, DP // 2:],
                    in_=wq_d["h"][DP // 2:].rearrange("t p i e -> p t i e"))
                nc.sync.dma_start(
                    out=wq_sb["l"],
                    in_=wq_d["l"].rearrange("t p i e -> p t i e"))
            for T in range(DP // 2):
                tl = xs_p.tile([128, 2, 2, CH], FP8, tag="xs")
                nc.sync.dma_start(out=tl, in_=xl_d[T, :, c, :, :, :])
                xl4.append(tl)
            xh = [xh4[t // 2][:, t % 2] for t in range(DP)]
            xl = [xl4[t // 2][:, t % 2] for t in range(DP)]
            rt_t = rt_p.tile([128, 2, CH], BF16, tag="rt")
            nc.sync.dma_start(out=rt_t,
                              in_=rt_d[:, :, cs].rearrange("t p s -> p t s"))
            if first:
                load_wk()

            def proj_qk(ps, w_sb, et, warm=False):
                """ps[128, CH] += x @ W[:, et-cols]: 3-term fp8 DR pairs,
                term-major so the first 8 matmuls need only hi planes.
                warm: interleave no-op matmuls so the PE stream has filler
                while the very first x/weight loads stream in."""
                n = 3 * DP
                i = 0
                for (wp, xs) in ((w_sb["h"], xh), (w_sb["l"], xh),
                                 (w_sb["h"], xl)):
                    for t in range(DP):
                        nc.tensor.matmul(
                            ps, wp[:, t, :, et * 128:(et + 1) * 128], xs[t],
                            start=(i == 0), stop=(i == n - 1), perf_mode=DR)
                        if warm:
                            warm_mm()
                        i += 1

            # ---------- q ----------
            rq_ps = ps_col.tile([128, 8], F32, tag="col")
            qr = []
            sqs = []
            for et in range(ET):
                q_ps = ps_big.tile([128, CH], F32, tag="big")
                proj_qk(q_ps, wq_sb, et)
                t = qr_p.tile([128, CH], BF16, tag="qr")
                # copy folds in the per-dim rmsnorm weight (w1/w2 by parity)
                nc.scalar.activation(out=t, in_=q_ps,
                                     func=mybir.ActivationFunctionType.Copy,
                                     bias=0.0,
                                     scale=wn_sb[:, et % 2:et % 2 + 1])
                qr.append(t)
                sqt = sq_p.tile([128, CH], BF16, tag="sq")
                nc.scalar.activation(out=sqt, in_=q_ps,
                                     func=mybir.ActivationFunctionType.Square,
                                     bias=0.0, scale=1.0)
                sqs.append(sqt)
            def q_colmms():
                for et in range(ET):
                    hh = et // 2
                    for j in range(4):
                        nc.tensor.matmul(
                            rq_ps[:, 4 * hh + j: 4 * hh + j + 1],
                            sqs[et][:, j * 128:(j + 1) * 128], ones_sb,
                            start=(et == 0 and j == 0),
                            stop=(et == ET - 1 and j == 3))
            q_colmms()

            qcs = slice((c % 2) * CH, (c % 2 + 1) * CH)   # qT ring slot

            def q_tail():
                # rotate first (only needs qr + tables), scale by the
                # broadcast rsqrt last — so the rope math never waits on the
                # NR/transpose/broadcast chain.  The scale path runs in bf16.
                us = []
                for hh in range(2):
                    u1 = qs_p.tile([128, CH], BF16, tag="qs",
                                   name=f"u1_{c}_{hh}")
                    u2 = qs_p.tile([128, CH], BF16, tag="qs",
                                   name=f"u2_{c}_{hh}")
                    _rope2(u1, u2, qr[2 * hh], qr[2 * hh + 1], rt_t)
                    us.append((u1, u2))
                rq_sb = nr_p.tile([128, 8], F32, tag="rq")
                nr_rsqrt(rq_sb, rq_ps, 8, scale16=False)
                rq_bf = nr_p.tile([128, 8], BF16, tag="rqb")
                nc.vector.tensor_copy(out=rq_bf, in_=rq_sb)
                t_row = col_to_row(rq_bf, 8)
                for hh in range(2):
                    rbt = rb_p.tile([128, CH], BF16, tag="rb")
                    nc.gpsimd.partition_broadcast(
                        rbt, t_row[0:1, hh * CH:(hh + 1) * CH])
                    nc.vector.tensor_mul(out=qT_sb[:, 2 * hh, qcs],
                                         in0=us[hh][0], in1=rbt)
                    nc.vector.tensor_mul(out=qT_sb[:, 2 * hh + 1, qcs],
                                         in0=us[hh][1], in1=rbt)
            with tc.high_priority():
                q_tail()   # latency-critical: scores of this chunk need qT
            if first:
                load_wv()

            # ---------- k ----------
            rk_ps = ps_col.tile([128, 8], F32, tag="col")
            kr = []
            ksqs = []
            for et in range(ET):
                k_ps = ps_big.tile([128, CH], F32, tag="big")
                proj_qk(k_ps, wk_sb, et)
                t = qr_p.tile([128, CH], BF16, tag="qr")
                nc.scalar.activation(out=t, in_=k_ps,
                                     func=mybir.ActivationFunctionType.Copy,
                                     bias=0.0,
                                     scale=wn_sb[:, 2 + et % 2:3 + et % 2])
                kr.append(t)
                sqt = sq_p.tile([128, CH], BF16, tag="sq")
                nc.scalar.activation(out=sqt, in_=k_ps,
                                     func=mybir.ActivationFunctionType.Square,
                                     bias=0.0, scale=1.0)
                ksqs.append(sqt)
            def k_colmms():
                for et in range(ET):
                    hh = et // 2
                    for j in range(4):
                        nc.tensor.matmul(
                            rk_ps[:, 4 * hh + j: 4 * hh + j + 1],
                            ksqs[et][:, j * 128:(j + 1) * 128], ones_sb,
                            start=(et == 0 and j == 0),
                            stop=(et == ET - 1 and j == 3))
            k_colmms()

            def k_tail():
                nr_rsqrt(rk_sb[:, :, 4 * c:4 * c + 4], rk_ps, 8,
                         scale16=True)
                for hh in range(2):
                    _rope(kT_sb, 2 * hh, cs, kr[2 * hh], kr[2 * hh + 1],
                          rt_t)
            with tc.high_priority():
                k_tail()   # latency-critical: scores need kT + rk scale

            # ---------- v ----------
            for st in range(4):
                v_ps = ps_big.tile([128, EL], F32, tag="big")
                n = 3 * DP
                i = 0
                for (x4s, wp) in ((xh4, wv_sb["h"]), (xl4, wv_sb["h"]),
                                  (xh4, wv_sb["l"])):
                    for t in range(DP):
                        T, tt = divmod(t, 2)
                        nc.tensor.matmul(
                            v_ps, x4s[T][:, tt, :, st * 128:(st + 1) * 128],
                            wp[:, t, :, :],
                            start=(i == 0), stop=(i == n - 1), perf_mode=DR)
                        i += 1
                # copy undoes the 64x weight prescale
                nc.scalar.activation(out=v_sb[:, 4 * c + st, :], in_=v_ps,
                                     func=mybir.ActivationFunctionType.Copy,
                                     bias=0.0, scale=1.0 / WSCALE)
            if first:
                load_msk()
            return lambda: None

        def phase_b(c, hh, aTh_t, aTl_t, filler=None):
            """Attention for (chunk c, head hh), software-pipelined: av/den
            matmuls for tile k are emitted after the scores matmuls for tile
            k+1 so the PE never waits in-order on exp(k)."""
            cs = slice(c * CH, (c + 1) * CH)
            n_sk = 4 * c + 4
            av_ps = {i: ps_big.tile([128, CH], F32, tag="big",
                                    name=f"av{c}_{hh}_{i}") for i in range(2)}
            den_ps = ps_col.tile([128, 4], F32, tag="col",
                                 name=f"den{c}_{hh}")

            def consume(e_t, skt, lo):
                first, last = (skt == 0), (skt == n_sk - 1)
                for half in range(2):
                    nc.tensor.matmul(
                        av_ps[half][:, lo:],
                        v_sb[:, skt, hh * 256 + half * 128:
                             hh * 256 + (half + 1) * 128],
                        e_t[:, lo:], start=first, stop=last)
                for j in range(lo // 128, 4):
                    nc.tensor.matmul(
                        den_ps[:, j:j + 1],
                        e_t[:, j * 128:(j + 1) * 128], ones_sb,
                        start=(first and j == 0), stop=(last and j == 3))

            from collections import deque
            pend = deque()
            for skt in range(n_sk):
                rel = skt - 4 * c
                # columns below 128*rel of a diagonal tile are fully masked
                lo = max(rel, 0) * 128
                sc_ps = ps_big.tile([128, CH], F32, tag="big",
                                    name=f"sc{c}_{hh}_{skt}")
                for half in range(2):
                    et = 2 * hh + half
                    nc.tensor.matmul(
                        sc_ps[:, lo:],
                        kT_sb[:, et, skt * 128:(skt + 1) * 128],
                        qT_sb[:, et, (c % 2) * CH + lo:(c % 2 + 1) * CH],
                        start=(half == 0), stop=(half == 1))
                e_t = e_p.tile([128, CH], BF16, tag="ep",
                               name=f"et{c}_{hh}_{skt}")
                nc.scalar.activation(out=e_t[:, lo:], in_=sc_ps[:, lo:],
                                     func=mybir.ActivationFunctionType.Exp,
                                     bias=0.0,
                                     scale=rk_sb[:, hh, skt:skt + 1])
                if rel >= 0:
                    nc.vector.tensor_mul(out=e_t[:, lo:], in0=e_t[:, lo:],
                                         in1=msk_sb[:, rel, lo:])
                if len(pend) >= 3:
                    consume(*pend.popleft())
                if filler is not None:
                    for fn in next(filler, []) or []:
                        fn()
                pend.append((e_t, skt, lo))
            while pend:
                consume(*pend.popleft())
            # copy av out of PSUM immediately so the psb ring slot frees fast
            avs = []
            for half in range(2):
                t = qs_p.tile([128, CH], BF16, tag="qs",
                              name=f"avs{c}_{hh}_{half}")
                nc.scalar.copy(out=t, in_=av_ps[half])
                avs.append(t)
            with tc.high_priority():
                dinv = nr_p.tile([128, 4], F32, tag="dinv",
                                 name=f"dinv{c}_{hh}")
                nc.vector.reciprocal(out=dinv, in_=den_ps)
                dinv_bf = nr_p.tile([128, 4], BF16, tag="dinvb",
                                    name=f"dinvb{c}_{hh}")
                nc.vector.tensor_copy(out=dinv_bf, in_=dinv)
                t_row = col_to_row(dinv_bf, 4)
                rd_t = rd_p.tile([128, CH], BF16, tag="rd", name=f"rd{c}_{hh}")
                nc.gpsimd.partition_broadcast(rd_t, t_row[0:1, 0:CH])
                for half in range(2):
                    et = 2 * hh + half
                    t = att_p.tile([128, CH], BF16, tag="att",
                                   name=f"att{c}_{hh}_{half}")
                    nc.vector.tensor_mul(out=t, in0=avs[half], in1=rd_t)
                    nc.scalar.copy(out=aTh_t[:, et, :], in_=t)
                    nc.vector.tensor_sub(out=aTl_t[:, et, :], in0=t,
                                         in1=aTh_t[:, et, :])

        def c_burst(c, aTh_t, aTl_t, dp):
            """Wo product for dout pair (2*dp, 2*dp+1), one bf16 store."""
            cs = slice(c * CH, (c + 1) * CH)
            o_t = o_p.tile([128, 2, CH], BF16, tag="op", name=f"ot{c}_{dp}")
            for j in range(2):
                dout = 2 * dp + j
                o_ps = ps_big.tile([128, CH], F32, tag="big",
                                   name=f"ops{c}_{dout}")
                i = 0
                for t in range(EP):
                    for (wp, ap) in ((wo_sb["h"], aTh_t), (wo_sb["l"], aTh_t),
                                     (wo_sb["h"], aTl_t)):
                        nc.tensor.matmul(
                            o_ps, wp[:, t, :, dout * 128:(dout + 1) * 128],
                            ap[:, 2 * t:2 * t + 2, :],
                            start=(i == 0), stop=(i == 3 * EP - 1),
                            perf_mode=DR)
                        i += 1
                if j == 0:
                    nc.vector.tensor_copy(out=o_t[:, j, :], in_=o_ps)
                else:
                    nc.scalar.copy(out=o_t[:, j, :], in_=o_ps)
            if c == 3:
                # last chunk: split dispatch between the HWDGE queues and the
                # Pool SWDGE queue so the drain isn't HWDGE-serialized; keep
                # the final bursts on the fast HWDGE queues
                eng = nc.gpsimd if dp < 3 else (
                    nc.sync if dp % 2 == 0 else nc.scalar)
            else:
                eng = nc.sync if dp % 2 == 0 else nc.scalar
            eng.dma_start(
                out=out_d[dp * 256:(dp + 1) * 256, cs].rearrange(
                    "(j p) s -> p j s", p=128),
                in_=o_t)

        def phase_c(c, aTh_t, aTl_t):
            for dp in range(DT // 2):
                c_burst(c, aTh_t, aTl_t, dp)

        def c_filler(c, aTh_t, aTl_t, lo, hi):
            for dp in range(lo, hi):
                yield [lambda c=c, ah=aTh_t, al=aTl_t, d=dp:
                       c_burst(c, ah, al, d)]

        # ---- schedule ----
        aTh = {}
        aTl = {}

        def new_aT(c):
            aTh[c] = at_p.tile([128, ET, CH], FP8, tag="ath", name=f"aTh{c}")
            aTl[c] = at_p.tile([128, ET, CH], FP8, tag="atl", name=f"aTl{c}")

        kt0 = phase_a(0, first=True)
        kt0()
        new_aT(0)
        phase_b(0, 0, aTh[0], aTl[0])
        kt1 = phase_a(1)
        kt1()
        phase_b(0, 1, aTh[0], aTl[0])
        load_wo()   # needed first by the chunk-0 Wo fillers in phase_b(1,*)
        def _delayed(gen, skip):
            for _ in range(skip):
                yield []
            yield from gen
        kt2 = phase_a(2)
        new_aT(1)
        phase_b(1, 0, aTh[1], aTl[1],
                filler=_delayed(c_filler(0, aTh[0], aTl[0], 0, 4), 3))
        phase_b(1, 1, aTh[1], aTl[1], filler=c_filler(0, aTh[0], aTl[0], 4, 8))
        kt2()
        kt3 = phase_a(3)
        new_aT(2)
        phase_b(2, 0, aTh[2], aTl[2],
                filler=_delayed(c_filler(1, aTh[1], aTl[1], 0, 4), 3))
        phase_b(2, 1, aTh[2], aTl[2], filler=c_filler(1, aTh[1], aTl[1], 4, 8))
        kt3()
        new_aT(3)
        phase_b(3, 0, aTh[3], aTl[3],
                filler=_delayed(c_filler(2, aTh[2], aTl[2], 0, 4), 8))
        phase_b(3, 1, aTh[3], aTl[3])
        for _d in range(4, 8):
            c_burst(2, aTh[2], aTl[2], _d)
        phase_c(3, aTh[3], aTl[3])


def _host_tables(position_ids, q_norm_w, k_norm_w):
    pos = np.asarray(position_ids).astype(np.float64)
    inv = 1.0 / (10000.0 ** (np.arange(0, HD, 2, dtype=np.float64) / HD))
    ang = pos[:, None] * inv[None, :]                      # [S, 128]
    rt = np.stack([np.cos(ang).T, np.sin(ang).T]).astype(NBF)  # [2, 128, S]
    qw = np.asarray(q_norm_w, np.float32)
    kw = np.asarray(k_norm_w, np.float32)
    wn = np.stack([qw[:HALF], qw[HALF:], kw[:HALF], kw[HALF:]],
                  axis=1).astype(np.float32)               # [128, 4]
    return rt, wn


def _host_masks():
    p = np.arange(HALF)[:, None]
    n = np.arange(CH)[None, :]
    return np.stack(
        [(rel * HALF + p <= n) for rel in range(4)]).astype(NBF)


def _pack_pair(mT):
    """[K, N] f32 -> (hi, lo) fp8 planes [K/256, 128, 2, N] for DoubleRow.

    plane[t, p, i, n] = m[256*t + 128*i + p, n]"""
    hi = mT.astype(NF8)
    lo = (mT - hi.astype(np.float32)).astype(NF8)

    def pk(a):
        k, n = a.shape
        return np.ascontiguousarray(
            a.reshape(k // 256, 2, 128, n).transpose(0, 2, 1, 3))
    return pk(hi), pk(lo)


def _pack_x(xT):
    """[D, S] f32 -> (hi, lo) fp8 [T, p, c, tt, i, s'] with
    element = xT[512*T + 256*tt + 128*i + p, c*CH + s'] (2KB contiguous
    per (T, p, c) line)."""
    hi = xT.astype(NF8)
    lo = (xT - hi.astype(np.float32)).astype(NF8)

    def pk(a):
        return np.ascontiguousarray(
            a.reshape(DP // 2, 2, 2, 128, NCH, CH).transpose(0, 3, 4, 1, 2, 5))
    return pk(hi), pk(lo)


def kernel(**inputs):
    x = np.asarray(inputs["x"], np.float32)
    sc = np.float32(WSCALE)
    Wq = np.asarray(inputs["Wq"], np.float32) * sc
    Wk = np.asarray(inputs["Wk"], np.float32) * sc
    Wv = np.asarray(inputs["Wv"], np.float32) * sc
    Wo = np.asarray(inputs["Wo"], np.float32) * sc
    rt, wn = _host_tables(inputs["position_ids"],
                          inputs["q_norm_w"], inputs["k_norm_w"])
    masks = _host_masks()

    if "nc" not in _CACHE:
        _CACHE["nc"] = _build()
    nc = _CACHE["nc"]
    ident = np.eye(128, dtype=np.float32)
    identb = np.eye(128, dtype=NBF)

    xpk = [_pack_x(np.ascontiguousarray(x[b].T)) for b in range(B)]
    in_maps = []
    for core in range(N_CORES):
        b, g = divmod(core, 4)
        es = slice(g * EL, (g + 1) * EL)
        wqh, wql = _pack_pair(np.ascontiguousarray(Wq[es, :].T))
        wkh, wkl = _pack_pair(np.ascontiguousarray(Wk[es, :].T))
        wvh, wvl = _pack_pair(np.ascontiguousarray(Wv[es, :].T))
        woh, wol = _pack_pair(np.ascontiguousarray(Wo[:, es].T))
        in_maps.append({
            "xh": xpk[b][0], "xl": xpk[b][1],
            "wqh": wqh, "wql": wql,
            "wkh": wkh, "wkl": wkl,
            "wvh": wvh, "wvl": wvl,
            "woh": woh, "wol": wol,
            "ident": ident, "identb": identb, "rt": rt, "wn": wn,
            "masks": masks,
        })
    res = run_bass_kernel_spmd(nc, in_maps, core_ids=list(range(N_CORES)))
    out = np.zeros((B, S, D), np.float32)
    inv = np.float32(1.0 / WSCALE)
    for core in range(N_CORES):
        b = core // 4
        out[b] += res.results[core]["outT"].astype(np.float32).T * inv
    return out
